# revision 1
# baseline (speedup 1.0000x reference)
"""Trainium2 Bass kernel for nn_DiffAttentionHead (B=16384, E=2048, H=16).

Key observation: the reference softmax is over a singleton key axis, so both
attention maps are identically 1.0 and the Q/K projections never affect the
output. The computation reduces, exactly, to
    x   = (1 - lam) * (value @ W_v.T)                  # (B, 1024)
    out = LayerNorm(x) * gamma + beta) @ W_o.T         # (B, 2048)
with lam a scalar from the lambda vectors.

Folds (host side): (1-lam) into W_v; gamma into W_o's rows; beta @ W_o.T into
a row vector r; and the LayerNorm itself into the second matmul's epilogue:
    out[b,e] = M[b,e]*inv[b] + (t[b]*R[e] + r[e]),  t = -mu*inv
where M = x @ woT on RAW x and R[e] = colsum(woT). So the device pipeline is
two f32r matmuls + PE transposes of x + per-row stats, with the LN applied as
two fused vector ops during PSUM eviction.

Sharding: data-parallel over batch across 8 NeuronCores (params replicated),
per the problem's sharding hint. No collectives needed.
"""
import numpy as np

import concourse.bacc as bacc
import concourse.mybir as mybir
import concourse.tile as tile
import concourse.masks as masks
from concourse.bass_utils import run_bass_kernel_spmd

F32 = mybir.dt.float32
F32R = mybir.dt.float32r
Alu = mybir.AluOpType
Act = mybir.ActivationFunctionType

B = 16384
E = 2048
J = 1024            # E // 2, the value/LN width
NCORES = 8
BLOC = B // NCORES  # rows per core
KT1 = E // 128      # MM1 contraction tiles
JT = J // 128       # MM2 contraction tiles
NBT = BLOC // 128   # 128-row tiles per core
N1 = 512
ET = E // 512
LN_EPS = 1e-5
LAG = 4             # phase2 pipeline lag (b-tiles)

_cache = {}


def _build(nbt=NBT, reps=1):
    nc = bacc.Bacc("TRN2", target_bir_lowering=False, debug=False, num_devices=NCORES)
    bloc = nbt * 128

    valT = nc.dram_tensor("valT", [E, bloc], F32R, kind="ExternalInput")
    wvT = nc.dram_tensor("wvT", [E, J], F32R, kind="ExternalInput")
    woT = nc.dram_tensor("woT", [J, E], F32R, kind="ExternalInput")
    rb = nc.dram_tensor("rb", [128, E], F32, kind="ExternalInput")
    Rb = nc.dram_tensor("Rb", [128, E], F32, kind="ExternalInput")
    out = nc.dram_tensor("out", [bloc, E], F32, kind="ExternalOutput")

    with tile.TileContext(nc) as tc:
        with tc.tile_pool(name="consts", bufs=1) as cpool, \
             tc.tile_pool(name="weights", bufs=1) as wpool, \
             tc.tile_pool(name="vals", bufs=2) as vpool, \
             tc.tile_pool(name="work", bufs=2) as kpool, \
             tc.tile_pool(name="psv", bufs=2, space="PSUM") as psv, \
             tc.tile_pool(name="pst", bufs=2, space="PSUM") as pst, \
             tc.tile_pool(name="pso", bufs=4, space="PSUM") as pso:

            ident_f = cpool.tile([128, 128], F32)
            masks.make_identity(nc, ident_f[:])
            ident = cpool.tile([128, 128], F32R)
            nc.vector.tensor_copy(ident[:], ident_f[:])
            eps_sb = cpool.tile([128, 1], F32)
            nc.vector.memset(eps_sb[:], LN_EPS)

            wv_all = wpool.tile([128, KT1 * J], F32R)
            wo_all = wpool.tile([128, JT * E], F32R)

            def emit_wo_half(half):
                lo = half * (JT // 2)
                nc.sync.dma_start(
                    out=wo_all[:, lo * E:(lo + JT // 2) * E].rearrange(
                        "p (t e) -> p t e", t=JT // 2),
                    in_=woT[lo * 128:(lo + JT // 2) * 128, :].rearrange(
                        "(t p) e -> p t e", p=128))

            def emit_wo_consts():
                rb_sb = cpool.tile([128, E], F32)
                nc.sync.dma_start(out=rb_sb[:], in_=rb[:])
                R_sb = cpool.tile([128, E], F32)
                nc.sync.dma_start(out=R_sb[:], in_=Rb[:])
                return rb_sb, R_sb

            state = {}
            stash = {}   # bt -> (x, s)
            lnts = {}    # bt -> transposed f32r tiles

            def transpose_tiles(bt):
                x, s = stash[bt]
                ps_t = pst.tile([128, 128], F32R, name="ps_t", tag="ps_t")
                jt = len(lnts.setdefault(bt, []))
                nc.tensor.transpose(ps_t[:], x[:, jt * 128:(jt + 1) * 128], ident[:])
                lt = kpool.tile([128, 128], F32R, name=f"lnt{jt}", tag=f"lnt{jt}",
                                bufs=LAG + 1)
                nc.vector.tensor_copy(lt[:], ps_t[:])
                lnts[bt].append(lt)

            def phase1(bt, first_rep):
                val_sb = vpool.tile([128, KT1 * 128], F32R, name="val_sb", tag="val_sb")
                if bt == 0 and first_rep:
                    # interleave per-k-tile val/wv loads so MM1 starts early
                    for t in range(KT1):
                        nc.sync.dma_start(
                            out=val_sb[:, t * 128:(t + 1) * 128],
                            in_=valT[t * 128:(t + 1) * 128, 0:128])
                        nc.sync.dma_start(
                            out=wv_all[:, t * J:(t + 1) * J],
                            in_=wvT[t * 128:(t + 1) * 128, :])
                else:
                    nc.sync.dma_start(
                        out=val_sb[:].rearrange("p (t b) -> p t b", t=KT1),
                        in_=valT[:, bt * 128:(bt + 1) * 128].rearrange(
                            "(t p) b -> p t b", p=128))

                x = kpool.tile([128, J], F32R, name="x", tag="x")
                s = kpool.tile([128, 8], F32, name="s", tag="s", bufs=LAG + 1)
                ps_vs = [psv.tile([128, N1], F32, name=f"ps_v{jc}", tag="ps_v")
                         for jc in range(J // N1)]
                for t in range(KT1):
                    for jc in range(J // N1):
                        nc.tensor.matmul(
                            ps_vs[jc][:],
                            val_sb[:, t * 128:(t + 1) * 128],
                            wv_all[:, t * J + jc * N1: t * J + (jc + 1) * N1],
                            start=(t == 0), stop=(t == KT1 - 1))
                    # interleave previous tile's transposes into the k-loop
                    if bt >= 1 and 2 <= t < 2 + JT:
                        transpose_tiles(bt - 1)
                for jc in range(J // N1):
                    nc.vector.tensor_scalar(
                        x[:, jc * N1:(jc + 1) * N1], ps_vs[jc][:], 1.0, 0.0,
                        op0=Alu.mult, op1=Alu.add, accum_out=s[:, jc:jc + 1])
                for jc in range(J // N1):
                    # x^2 on ACT; output into the dead val tile, keep only accum
                    nc.scalar.activation(
                        val_sb[:, jc * N1:(jc + 1) * N1],
                        x[:, jc * N1:(jc + 1) * N1], Act.Square,
                        accum_out=s[:, 2 + jc:3 + jc])
                # stats: s4=mu s5=var s6=inv s7=-mu*inv
                nc.vector.tensor_scalar(s[:, 4:5], s[:, 0:1], s[:, 1:2], 1.0 / J,
                                        op0=Alu.add, op1=Alu.mult)
                nc.vector.tensor_scalar(s[:, 5:6], s[:, 2:3], s[:, 3:4], 1.0 / J,
                                        op0=Alu.add, op1=Alu.mult)
                nc.vector.scalar_tensor_tensor(
                    s[:, 0:1], s[:, 4:5], -1.0, s[:, 4:5], op0=Alu.mult, op1=Alu.mult)
                nc.vector.tensor_tensor(s[:, 5:6], s[:, 5:6], s[:, 0:1], op=Alu.add)
                nc.scalar.activation(s[:, 6:7], s[:, 5:6], Act.Sqrt,
                                     bias=eps_sb[:], scale=1.0)
                nc.vector.reciprocal(s[:, 6:7], s[:, 6:7])
                nc.vector.scalar_tensor_tensor(
                    s[:, 7:8], s[:, 4:5], -1.0, s[:, 6:7], op0=Alu.mult, op1=Alu.mult)
                stash[bt] = (x, s)

            def phase2(bt):
                x, s = stash.pop(bt)
                lnt = lnts.pop(bt)
                rb_sb, R_sb = state["wo"]
                ps_os = [pso.tile([128, 512], F32, name=f"ps_o{ec}", tag="ps_o")
                         for ec in range(ET)]
                for jt in range(JT):
                    for ec in range(ET):
                        nc.tensor.matmul(
                            ps_os[ec][:], lnt[jt][:],
                            wo_all[:, jt * E + ec * 512: jt * E + (ec + 1) * 512],
                            start=(jt == 0), stop=(jt == JT - 1))
                for ec in range(ET):
                    F = kpool.tile([128, 512], F32, name="F", tag="F", bufs=3)
                    # F = t*R + r
                    nc.vector.scalar_tensor_tensor(
                        F[:], R_sb[:, ec * 512:(ec + 1) * 512], s[:, 7:8],
                        rb_sb[:, ec * 512:(ec + 1) * 512],
                        op0=Alu.mult, op1=Alu.add)
                    # O = M*inv + F  (also the PSUM eviction; in place into F)
                    nc.vector.scalar_tensor_tensor(
                        F[:], ps_os[ec][:], s[:, 6:7], F[:],
                        op0=Alu.mult, op1=Alu.add)
                    nc.sync.dma_start(
                        out=out[bt * 128:(bt + 1) * 128, ec * 512:(ec + 1) * 512],
                        in_=F[:])

            first = True
            for rep in range(reps):
                for bt in range(nbt + LAG):
                    if bt < nbt:
                        phase1(bt, first_rep=(rep == 0))
                    if first and bt == 2:
                        emit_wo_half(0)
                    if first and bt == 3:
                        emit_wo_half(1)
                        state["wo"] = emit_wo_consts()
                        first = False
                    if bt == nbt:
                        while len(lnts.setdefault(nbt - 1, [])) < JT:
                            transpose_tiles(nbt - 1)
                    if bt >= LAG:
                        phase2(bt - LAG)

    nc.compile()
    return nc


def _prep_inputs(value, W_v, W_o, lambda_q1, lambda_k1, lambda_q2, lambda_k2,
                 ln_gamma, ln_beta):
    value = np.asarray(value, dtype=np.float32)
    W_v = np.asarray(W_v, dtype=np.float32)
    W_o = np.asarray(W_o, dtype=np.float32)
    lq1 = np.asarray(lambda_q1, dtype=np.float64)
    lk1 = np.asarray(lambda_k1, dtype=np.float64)
    lq2 = np.asarray(lambda_q2, dtype=np.float64)
    lk2 = np.asarray(lambda_k2, dtype=np.float64)
    gamma = np.asarray(ln_gamma, dtype=np.float32)
    beta = np.asarray(ln_beta, dtype=np.float32)

    lam = np.exp((lq1 * lk1).sum()) - np.exp((lq2 * lk2).sum()) + 0.8
    c = np.float32(1.0 - lam)

    wvT = np.ascontiguousarray((c * W_v).T)                      # (E, J)
    woT = np.ascontiguousarray(W_o.T * gamma[:, None])           # (J, E)
    r = (W_o.astype(np.float64) @ beta.astype(np.float64)).astype(np.float32)
    R = woT.astype(np.float64).sum(axis=0).astype(np.float32)
    rbc = np.ascontiguousarray(np.broadcast_to(r, (128, E)))
    Rbc = np.ascontiguousarray(np.broadcast_to(R, (128, E)))

    in_maps = []
    for cidx in range(NCORES):
        shard = value[cidx * BLOC:(cidx + 1) * BLOC]
        valT = np.ascontiguousarray(shard.T)
        in_maps.append({"valT": valT, "wvT": wvT, "woT": woT,
                        "rb": rbc, "Rb": Rbc})
    return in_maps


def kernel(query, key, value, W_q, W_k, W_v, W_o,
           lambda_q1, lambda_k1, lambda_q2, lambda_k2,
           ln_gamma, ln_beta):
    """Full-input entry point: shards over 8 NeuronCores internally and
    returns the full (B, E) float32 output."""
    in_maps = _prep_inputs(value, W_v, W_o, lambda_q1, lambda_k1,
                           lambda_q2, lambda_k2, ln_gamma, ln_beta)
    if "nc" not in _cache:
        _cache["nc"] = _build()
    nc = _cache["nc"]
    res = run_bass_kernel_spmd(nc, in_maps, list(range(NCORES)))
    return np.concatenate([res.results[c]["out"] for c in range(NCORES)], axis=0)

